# revision 5
# baseline (speedup 1.0000x reference)
"""Trainium2 Bass kernel for nn_MessageBlock (GNN message passing).

Strategy (8 NeuronCores, SPMD):
  - Destination-partition nodes across cores: core k owns dst nodes
    [k*NN, (k+1)*NN). All edges with dst in that range are processed on k.
  - Host pre-sorts edges by (destination window, j>=32768) and lays out all
    per-edge data (radial^T, f_cut, unit vectors, dst slot, gather indices)
    into fixed-budget padded tile layouts so all 8 cores run one identical
    static program.
  - Device phase A: phi = silu(s@W1+b1)@W2+b2 for all nodes (computed
    replicated per core) into a DRAM table.
  - Device phase B: per 2-window "super": dma_gather phi rows and v rows for
    the super's edges (two gathers each: table split at 32768 rows since
    gather indices are int16), compute per-edge messages
    msg = [x_s | v*x_vv + x_vs*u], and segment-sum via one-hot matmuls
    accumulating in PSUM per 125-node window; flush adds s|v and stores.
  - Host reassembles [N,F] and [N,3,F] outputs from per-core window slabs.

No collectives needed: each core's outputs are disjoint node ranges.
"""

import numpy as np

import concourse.bacc as bacc
import concourse.tile as tile
from concourse import mybir
from concourse.bass_utils import run_bass_kernel_spmd
from concourse.library_config import mlp

F32 = mybir.dt.float32
F32R = mybir.dt.float32r
I16 = mybir.dt.int16
AF = mybir.ActivationFunctionType

NC_CORES = 8
WIN = 125          # dst slots per window (<=128 for one-hot matmul M dim)
P = 128


def _round_up(x, m):
    return (x + m - 1) // m * m


def _make_cfg(N, E, F, R, edge_index):
    NN = N // NC_CORES
    assert N % NC_CORES == 0 and NN % WIN == 0
    NW = NN // WIN
    assert NW % 2 == 0
    NSUP = NW // 2
    NPT = (N + P - 1) // P          # node tiles
    NPAIR = (NPT + 1) // 2
    TROWS = NPAIR * 256             # phi table rows (padded)
    LO_ROWS = min(TROWS, 32768)
    HI_OFF = LO_ROWS if TROWS > LO_ROWS else 0
    HI_ROWS = max(TROWS - HI_OFF, 256)

    i64 = np.asarray(edge_index[0]).astype(np.int64)
    j64 = np.asarray(edge_index[1]).astype(np.int64)
    core = i64 // NN
    loc = i64 - core * NN
    w = loc // WIN
    gw = core * NW + w
    ishi = (j64 >= LO_ROWS).astype(np.int64)
    key = gw * 2 + ishi
    cnt = np.bincount(key, minlength=2 * NC_CORES * NW)
    n_lo = cnt[0::2].max() if E else P
    n_hi = cnt[1::2].max() if E else P
    W_LO = max(_round_up(int(n_lo), P), P)
    W_HI = max(_round_up(int(n_hi), P), P)
    KLO = W_LO // P
    KHI = W_HI // P
    TS = 2 * (KLO + KHI)
    return dict(
        N=N, E=E, F=F, R=R, NN=NN, NW=NW, NSUP=NSUP, NPT=NPT, NPAIR=NPAIR,
        TROWS=TROWS, LO_ROWS=LO_ROWS, HI_OFF=HI_OFF, HI_ROWS=HI_ROWS,
        W_LO=W_LO, W_HI=W_HI, KLO=KLO, KHI=KHI, TS=TS,
        IW=(2 * W_LO + 2 * W_HI) // 16,
        i64=i64, j64=j64, core=core, w=w, slot=loc - w * WIN, key=key,
        ishi=ishi.astype(bool),
    )


def _host_prep(cfg, s, v, radial, f_cut, unit):
    N, E = cfg["N"], cfg["E"]
    NN, NW, NSUP = cfg["NN"], cfg["NW"], cfg["NSUP"]
    W_LO, W_HI, KLO, KHI, TS = (
        cfg["W_LO"], cfg["W_HI"], cfg["KLO"], cfg["KHI"], cfg["TS"]
    )
    LO_ROWS = cfg["LO_ROWS"]

    key = cfg["key"]
    order = np.argsort(key, kind="stable")
    ks = key[order]
    counts = np.bincount(key, minlength=2 * NC_CORES * NW)
    starts = np.concatenate([[0], np.cumsum(counts)[:-1]])
    rank = np.arange(E, dtype=np.int64) - starts[ks]

    core_s = cfg["core"][order]
    w_s = cfg["w"][order]
    slot_s = cfg["slot"][order]
    j_s = cfg["j64"][order]
    hi_s = cfg["ishi"][order]
    sup = w_s // 2
    w01 = w_s % 2

    rpos = np.where(hi_s, w01 * W_HI + rank, w01 * W_LO + rank)
    st = np.where(hi_s, 2 * KLO + rpos // P, rpos // P).astype(np.int64)
    pp = (rpos % P).astype(np.int64)
    assert (np.where(hi_s, rank < W_HI, rank < W_LO)).all()

    meta = np.zeros((NC_CORES, NSUP, P, TS * 8), np.float32)
    meta[core_s, sup, pp, st * 8 + 0] = np.asarray(f_cut)[order]
    un = np.asarray(unit)[order]
    for c in range(3):
        meta[core_s, sup, pp, st * 8 + 1 + c] = un[:, c]
    meta[core_s, sup, pp, st * 8 + 4] = slot_s.astype(np.float32)

    R = cfg["R"]
    radT = np.zeros((NC_CORES, NSUP, 64, TS // 2, P), np.float32)
    radT[
        core_s[:, None], sup[:, None],
        (st % 2)[:, None] * 32 + np.arange(R)[None, :],
        (st // 2)[:, None], pp[:, None],
    ] = np.asarray(radial)[order]

    lov = np.zeros((NC_CORES, NSUP, 2 * W_LO), np.int16)
    hiv = np.zeros((NC_CORES, NSUP, 2 * W_HI), np.int16)
    lo_m = ~hi_s
    lov[core_s[lo_m], sup[lo_m], (w01 * W_LO + rank)[lo_m]] = j_s[lo_m]
    hiv[core_s[hi_s], sup[hi_s], (w01 * W_HI + rank)[hi_s]] = (
        j_s[hi_s] - LO_ROWS
    )

    def wrap16(a):
        nn = a.shape[-1]
        b = a.reshape(NC_CORES, NSUP, nn // 16, 16).transpose(0, 1, 3, 2)
        return np.tile(b, (1, 1, 8, 1))

    idxb = np.concatenate([wrap16(lov), wrap16(hiv)], axis=-1)

    sv = np.zeros((NC_CORES, NW, P, 256), np.float32)
    nodes = np.arange(N, dtype=np.int64)
    core_n = nodes // NN
    loc_n = nodes - core_n * NN
    w_n = loc_n // WIN
    p_n = loc_n - w_n * WIN
    sv[core_n, w_n, p_n, :64] = np.asarray(s)
    sv[core_n, w_n, p_n, 64:] = np.asarray(v).reshape(N, 192)

    sT = np.zeros((64, cfg["TROWS"]), np.float32)
    sT[:, :N] = np.asarray(s).T

    vtab = np.zeros((cfg["TROWS"], 192), np.float32)
    vtab[:N] = np.asarray(v).reshape(N, 192)

    return dict(meta=meta, radT=radT, idxb=idxb, sv=sv, sT=sT, vtab=vtab)


def _build(cfg, has_b1, has_b2, has_br):
    NSUP, NW, TS = cfg["NSUP"], cfg["NW"], cfg["TS"]
    KLO, KHI = cfg["KLO"], cfg["KHI"]
    W_LO, W_HI, IW = cfg["W_LO"], cfg["W_HI"], cfg["IW"]
    NPAIR, TROWS = cfg["NPAIR"], cfg["TROWS"]
    LO_ROWS, HI_OFF, HI_ROWS = cfg["LO_ROWS"], cfg["HI_OFF"], cfg["HI_ROWS"]

    nc = bacc.Bacc(
        "TRN2", target_bir_lowering=False, debug=False, num_devices=NC_CORES
    )
    d_sT = nc.dram_tensor("sT", [64, TROWS], F32R, kind="ExternalInput")
    d_w1 = nc.dram_tensor("w1", [64, 64], F32R, kind="ExternalInput")
    d_w2 = nc.dram_tensor("w2p", [64, 256], F32R, kind="ExternalInput")
    d_wr = nc.dram_tensor("wrp", [64, 256], F32R, kind="ExternalInput")
    d_b1 = nc.dram_tensor("b1c", [64, 1], F32, kind="ExternalInput")
    d_vt = nc.dram_tensor("vtab", [TROWS, 192], F32, kind="ExternalInput")
    d_rad = nc.dram_tensor(
        "radT", [NSUP, 64, (TS // 2) * P], F32R, kind="ExternalInput"
    )
    d_meta = nc.dram_tensor(
        "meta", [NSUP, P, TS * 8], F32, kind="ExternalInput"
    )
    d_idx = nc.dram_tensor("idxb", [NSUP, P, IW], I16, kind="ExternalInput")
    d_sv = nc.dram_tensor("sv", [NW, P, 256], F32, kind="ExternalInput")
    d_iota = nc.dram_tensor("iotam", [P, P], F32, kind="ExternalInput")
    d_out = nc.dram_tensor("out_sv", [NW, P, 256], F32, kind="ExternalOutput")
    d_phi = nc.dram_tensor("phi", [TROWS, 192], F32)
    d_b2 = d_br = None
    if has_b2:
        d_b2 = nc.dram_tensor("b2bc", [P, 192], F32, kind="ExternalInput")
    if has_br:
        d_br = nc.dram_tensor("brbc", [P, 192], F32, kind="ExternalInput")

    nc.gpsimd.load_library(mlp)

    with tile.TileContext(nc) as tc:
        with (
            tc.tile_pool(name="static", bufs=1) as static,
        ):
            w1_t = static.tile([64, 64], F32R)
            nc.sync.dma_start(w1_t[:], d_w1[:])
            w2_t = static.tile([64, 256], F32R)
            nc.sync.dma_start(w2_t[:], d_w2[:])
            wr_t = static.tile([64, 256], F32R)
            nc.sync.dma_start(wr_t[:], d_wr[:])
            b1_t = static.tile([64, 1], F32)
            nc.sync.dma_start(b1_t[:], d_b1[:])
            iota_t = static.tile([P, P], F32)
            nc.sync.dma_start(iota_t[:], d_iota[:])
            b2_t = br_t = None
            if has_b2:
                b2_t = static.tile([P, 192], F32)
                nc.sync.dma_start(b2_t[:], d_b2[:])
            if has_br:
                br_t = static.tile([P, 192], F32)
                nc.sync.dma_start(br_t[:], d_br[:])

            # ---------------- phase A: phi table ----------------
            CH = 32  # node-tile pairs per chunk
            with (
                tc.tile_pool(name="schunk", bufs=2) as schunk,
                tc.tile_pool(name="phiw", bufs=4) as phiw,
                tc.tile_pool(name="ph1p", bufs=2, space="PSUM") as ph1p,
                tc.tile_pool(name="ph2p", bufs=3, space="PSUM") as ph2p,
            ):
                for c0 in range(0, NPAIR, CH):
                    c1 = min(c0 + CH, NPAIR)
                    cols = (c1 - c0) * 256
                    sch = schunk.tile([64, cols], F32R, name="sch")
                    nc.sync.dma_start(sch[:], d_sT[:, c0 * 256:c0 * 256 + cols])
                    for u in range(c0, c1):
                        off = (u - c0) * 256
                        psum1 = ph1p.tile([64, 256], F32, name="psum1")
                        nc.tensor.matmul(
                            psum1[:], w1_t[:], sch[:, off:off + 256],
                            start=True, stop=True,
                        )
                        zt = phiw.tile([64, 256], F32, name="zt")
                        if has_b1:
                            nc.scalar.activation(
                                zt[:], psum1[:], AF.Identity, bias=b1_t[:]
                            )
                        else:
                            nc.scalar.activation(zt[:], psum1[:], AF.Copy)
                        sig = phiw.tile([64, 256], F32, name="sig")
                        nc.scalar.activation(sig[:], zt[:], AF.Sigmoid)
                        ph1 = phiw.tile([64, 256], F32R, name="ph1")
                        nc.vector.tensor_mul(ph1[:], zt[:], sig[:])
                        for sub in range(2):
                            psum2 = ph2p.tile([P, 256], F32, name="psum2")
                            nc.tensor.matmul(
                                psum2[:], ph1[:, sub * P:(sub + 1) * P],
                                w2_t[:], start=True, stop=True,
                            )
                            pho = phiw.tile([P, 192], F32, name="pho")
                            if has_b2:
                                nc.vector.tensor_add(
                                    pho[:], psum2[:, :192], b2_t[:]
                                )
                            else:
                                nc.scalar.activation(
                                    pho[:], psum2[:, :192], AF.Copy
                                )
                            row0 = (2 * u + sub) * P
                            nc.sync.dma_start(
                                d_phi[row0:row0 + P, :], pho[:]
                            )

            # ---------------- phase B: edges ----------------
            with (
                tc.tile_pool(name="slabs", bufs=2) as slabs,
                tc.tile_pool(name="small", bufs=2) as small,
                tc.tile_pool(name="work", bufs=4) as work,
                tc.tile_pool(name="wp", bufs=4, space="PSUM") as wpp,
                tc.tile_pool(name="accp", bufs=2, space="PSUM") as accp,
            ):
                for sidx in range(NSUP):
                    idx_t = small.tile([P, IW], I16, name="idx_t")
                    nc.sync.dma_start(idx_t[:], d_idx[sidx])
                    rad_t = small.tile([64, (TS // 2) * P], F32R, name="rad_t")
                    nc.sync.dma_start(rad_t[:], d_rad[sidx])
                    meta_t = small.tile([P, TS * 8], F32, name="meta_t")
                    nc.sync.dma_start(meta_t[:], d_meta[sidx])

                    slab_p = slabs.tile([P, TS, 192], F32, name="slab_p")
                    slab_v = slabs.tile([P, TS, 192], F32, name="slab_v")
                    nc.gpsimd.dma_gather(
                        slab_p[:, 0:2 * KLO, :], d_phi[0:LO_ROWS, :],
                        idx_t[:, 0:2 * W_LO // 16], 2 * W_LO, 2 * W_LO, 192,
                        single_packet=False,
                    )
                    nc.gpsimd.dma_gather(
                        slab_p[:, 2 * KLO:TS, :],
                        d_phi[HI_OFF:HI_OFF + HI_ROWS, :],
                        idx_t[:, 2 * W_LO // 16:IW], 2 * W_HI, 2 * W_HI, 192,
                        single_packet=False,
                    )
                    nc.gpsimd.dma_gather(
                        slab_v[:, 0:2 * KLO, :], d_vt[0:LO_ROWS, :],
                        idx_t[:, 0:2 * W_LO // 16], 2 * W_LO, 2 * W_LO, 192,
                        single_packet=False,
                    )
                    nc.gpsimd.dma_gather(
                        slab_v[:, 2 * KLO:TS, :],
                        d_vt[HI_OFF:HI_OFF + HI_ROWS, :],
                        idx_t[:, 2 * W_LO // 16:IW], 2 * W_HI, 2 * W_HI, 192,
                        single_packet=False,
                    )

                    for w01 in range(2):
                        wdx = 2 * sidx + w01
                        sv_t = work.tile([P, 256], F32, name="sv_t")
                        nc.sync.dma_start(sv_t[:], d_sv[wdx])
                        acc = accp.tile([P, 256], F32, name="acc")
                        sts = (
                            [w01 * KLO + t for t in range(KLO)]
                            + [2 * KLO + w01 * KHI + t for t in range(KHI)]
                        )
                        for ti, st in enumerate(sts):
                            phi_t = slab_p[:, st, :]
                            v_t = slab_v[:, st, :]
                            u_, sub = st // 2, st % 2
                            radsl = rad_t[
                                32 * sub:32 * sub + 32, u_ * P:(u_ + 1) * P
                            ]
                            fcut_ap = meta_t[:, st * 8:st * 8 + 1]
                            u3 = meta_t[:, st * 8 + 1:st * 8 + 4]
                            slot_ap = meta_t[:, st * 8 + 4:st * 8 + 5]

                            wp = wpp.tile([P, 256], F32, name="wp")
                            nc.tensor.matmul(
                                wp[:], radsl,
                                wr_t[32 * sub:32 * sub + 32, :],
                                start=True, stop=True,
                            )
                            wsc = work.tile([P, 192], F32, name="wsc")
                            if has_br:
                                wtmp = work.tile([P, 192], F32, name="wtmp")
                                nc.vector.tensor_add(
                                    wtmp[:], wp[:, :192], br_t[:]
                                )
                                nc.scalar.activation(
                                    wsc[:], wtmp[:], AF.Copy, scale=fcut_ap
                                )
                            else:
                                nc.scalar.activation(
                                    wsc[:], wp[:, :192], AF.Copy, scale=fcut_ap
                                )

                            msg = work.tile([P, 256], F32R, name="msg")
                            xvv = work.tile([P, 64], F32, name="xvv")
                            xvs = work.tile([P, 64], F32, name="xvs")
                            nc.vector.tensor_mul(
                                msg[:, 0:64], phi_t[:, 0:64], wsc[:, 0:64]
                            )
                            nc.vector.tensor_mul(
                                xvv[:], phi_t[:, 64:128], wsc[:, 64:128]
                            )
                            nc.vector.tensor_mul(
                                xvs[:], phi_t[:, 128:192], wsc[:, 128:192]
                            )
                            # msg[64:256] = v * xvv3
                            nc.vector.tensor_mul(
                                msg[:, 64:256].rearrange(
                                    "p (c f) -> p c f", c=3
                                ),
                                v_t.rearrange("p (c f) -> p c f", c=3),
                                xvv[:, None, :].to_broadcast([P, 3, 64]),
                            )
                            tmp = work.tile([P, 192], F32, name="tmp")
                            for c in range(3):
                                nc.scalar.activation(
                                    tmp[:, c * 64:(c + 1) * 64], xvs[:],
                                    AF.Copy, scale=u3[:, c:c + 1],
                                )
                            nc.vector.tensor_add(
                                msg[:, 64:256], msg[:, 64:256], tmp[:]
                            )
                            oh = work.tile([P, P], F32R, name="oh")
                            nc.vector.tensor_scalar(
                                oh[:], iota_t[:], slot_ap, None,
                                mybir.AluOpType.is_equal,
                            )
                            nc.tensor.matmul(
                                acc[:], oh[:], msg[:],
                                start=(ti == 0), stop=(ti == len(sts) - 1),
                            )
                        acc_sb = work.tile([P, 256], F32, name="acc_sb")
                        nc.vector.tensor_add(acc_sb[:], acc[:], sv_t[:])
                        nc.sync.dma_start(d_out[wdx], acc_sb[:])
    nc.compile()
    return nc


def _assemble(cfg, results):
    N, NN, NW = cfg["N"], cfg["NN"], cfg["NW"]
    s_out = np.empty((N, 64), np.float32)
    v_out = np.empty((N, 3, 64), np.float32)
    for k in range(NC_CORES):
        o = results[k]["out_sv"]  # [NW, 128, 256]
        o = o[:, :WIN, :].reshape(NN, 256)
        s_out[k * NN:(k + 1) * NN] = o[:, :64]
        v_out[k * NN:(k + 1) * NN] = o[:, 64:].reshape(NN, 3, 64)
    return s_out, v_out


def prepare(inputs):
    """Host prep + program build. Returns (nc, cfg, in_maps)."""
    s = np.asarray(inputs["s"], np.float32)
    v = np.asarray(inputs["v"], np.float32)
    radial = np.asarray(inputs["radial_embeddings"], np.float32)
    f_cut = np.asarray(inputs["f_cut"], np.float32)
    unit = np.asarray(inputs["unit_vectors"], np.float32)
    ei = np.asarray(inputs["edge_index"])
    W1 = np.asarray(inputs["W1"], np.float32)
    b1 = np.asarray(inputs["b1"], np.float32)
    W2 = np.asarray(inputs["W2"], np.float32)
    b2 = np.asarray(inputs["b2"], np.float32)
    Wr = np.asarray(inputs["Wr"], np.float32)
    br = np.asarray(inputs["br"], np.float32)

    N, F = s.shape
    E = ei.shape[1]
    R = radial.shape[1]
    cfg = _make_cfg(N, E, F, R, ei)
    arrs = _host_prep(cfg, s, v, radial, f_cut, unit)

    has_b1 = bool(np.any(b1))
    has_b2 = bool(np.any(b2))
    has_br = bool(np.any(br))
    nc = _build(cfg, has_b1, has_b2, has_br)

    w2p = np.zeros((64, 256), np.float32)
    w2p[:, :192] = W2
    wrp = np.zeros((64, 256), np.float32)
    wrp[:32, :192] = Wr
    wrp[32:, :192] = Wr

    in_maps = []
    for k in range(NC_CORES):
        m = dict(
            sT=arrs["sT"], w1=W1, w2p=w2p, wrp=wrp,
            b1c=b1.reshape(64, 1), vtab=arrs["vtab"],
            radT=arrs["radT"][k].reshape(cfg["NSUP"], 64, -1),
            meta=arrs["meta"][k], idxb=arrs["idxb"][k], sv=arrs["sv"][k],
            iotam=np.tile(np.arange(P, dtype=np.float32)[None, :], (P, 1)),
        )
        if has_b2:
            m["b2bc"] = np.tile(b2[None, :], (P, 1))
        if has_br:
            m["brbc"] = np.tile(br[None, :], (P, 1))
        in_maps.append(m)
    return nc, cfg, in_maps


def kernel(**inputs):
    nc, cfg, in_maps = prepare(inputs)
    res = run_bass_kernel_spmd(nc, in_maps, core_ids=list(range(NC_CORES)))
    return _assemble(cfg, res.results)
